# revision 2
# baseline (speedup 1.0000x reference)
"""Trainium2 Bass kernel for DNAS PreBasicBlock (mixed-quantization residual block).

Math:
  out = residual + mixed_qconv2(bn_relu2(mixed_qconv1(bn_relu1(x))))

Key optimizations:
  * relu+clip fold: bn_relu followed by clip(.,0,1) == clip(bn(.),0,1); quantized
    activations A_n = clamp(round(n*bn(x)), 0, n) are small integers, exact in bf16.
  * candidate folding: the 9-way weighted conv sum groups by activation bits:
    out = sum_g conv(A_g, W_g) with W_g = sum_{bw} p_k * wq_k / n_g, so only
    3 convs per layer instead of 9.
  * A_4 = round(A_8/17) and A_2 = round(A_4/5) hold exactly, so only the 8-bit
    grid is stored; 4/2-bit grids are derived per-superchunk on the fly.
  * weights split hi/lo bf16 (W ~ hi+lo to ~2^-18 relative) -> conv runs at bf16
    PE rate with near-fp32 accuracy; activations are exact small ints in bf16.
  * conv as 9 shift-matmuls per (group, half) accumulating in PSUM; data-parallel
    over batch across 8 cores; BN batch stats via two tiny (1KB) AllReduces.
"""
import sys

sys.path.insert(0, "/opt/trn_rl_repo")

import numpy as np

import concourse.bass as bass
import concourse.tile as tile
from concourse import bacc, bass_isa, bass_utils, mybir

dt = mybir.dt
Alu = mybir.AluOpType
Act = mybir.ActivationFunctionType

N_CORES = 8
B, C, H, W = 32, 128, 56, 56
BS = B // N_CORES          # batch shard per core
HP, WP = H + 2, W + 4      # padded image: 1 row top/bottom, 2 cols left/right
IMG = HP * WP              # 3480
APIX = BS * IMG            # 13920
BASE = WP + 2              # first valid flat offset within an image (row 1, col 2)
VSTART = BASE              # 62
VEND = (BS - 1) * IMG + (H * WP + W + 1) + 1  # one past last valid: 13858
CHUNK = 512
NCHUNK = -(-(VEND - VSTART) // CHUNK)  # 27
SCCH = 3                   # chunks per superchunk
SLEN = SCCH * CHUNK        # 1536
NSC = -(-NCHUNK // SCCH)   # 9
STG = 1664                 # derive-staging width (halo 64 each side)
NPIX_IMG = H * W           # 3136
IMG_SPAN = H * WP          # 3360: rows 1..56 as 56 x 60 view
NTOT = float(B * H * W)    # BN divisor 100352
MAGIC = 12582912.0         # 1.5*2^23: fp32 round-to-int via add/sub
MAGICB = 192.0             # 1.5*2^7: bf16 round-to-int via f32->bf16 convert
EPS = 1e-5

BITS = [2, 4, 8]
NW = [2 ** BITS[k // 3] - 1 for k in range(9)]   # weight levels per candidate
NA = [2 ** BITS[k % 3] - 1 for k in range(9)]    # activation levels per candidate
KORDER = [2, 5, 8, 1, 4, 7, 0, 3, 6]             # group-major, ba=8 group first
TAPS = [(ky - 1) * WP + (kx - 1) for ky in range(3) for kx in range(3)]

_CACHE = {}


def _chunks_of_sc(s):
    """[(psum_col, global_start, length), ...] for superchunk s."""
    out = []
    for c in range(SCCH):
        ci = s * SCCH + c
        if ci >= NCHUNK:
            break
        gs = VSTART + ci * CHUNK
        ln = min(CHUNK, VEND - gs)
        out.append((c * CHUNK, gs, ln))
    return out


def _build():
    nc = bacc.Bacc("TRN2", target_bir_lowering=False, debug=False,
                   num_devices=N_CORES)

    x_in = nc.dram_tensor("x", [BS, C, H, W], dt.float32, kind="ExternalInput")
    w1_in = nc.dram_tensor("conv1_w", [9, C, C, 3, 3], dt.float32, kind="ExternalInput")
    w2_in = nc.dram_tensor("conv2_w", [9, C, C, 3, 3], dt.float32, kind="ExternalInput")
    g1_in = nc.dram_tensor("gamma1", [C], dt.float32, kind="ExternalInput")
    b1_in = nc.dram_tensor("beta1", [C], dt.float32, kind="ExternalInput")
    g2_in = nc.dram_tensor("gamma2", [C], dt.float32, kind="ExternalInput")
    b2_in = nc.dram_tensor("beta2", [C], dt.float32, kind="ExternalInput")
    p1_in = nc.dram_tensor("p1", [9], dt.float32, kind="ExternalInput")
    p2_in = nc.dram_tensor("p2", [9], dt.float32, kind="ExternalInput")
    gn1_in = nc.dram_tensor("gn1", [9], dt.float32, kind="ExternalInput")
    gn2_in = nc.dram_tensor("gn2", [9], dt.float32, kind="ExternalInput")
    tau_in = nc.dram_tensor("tau", [1], dt.float32, kind="ExternalInput")
    consts_in = nc.dram_tensor("consts", [1, 18], dt.float32, kind="ExternalInput")
    out_dram = nc.dram_tensor("out", [BS, C, H, W], dt.float32, kind="ExternalOutput")

    from concourse.masks import make_identity

    with tile.TileContext(nc) as tc:
        with tc.tile_pool(name="main", bufs=1) as sb, \
             tc.tile_pool(name="ps", bufs=1, space="PSUM") as ps, \
             tc.tile_pool(name="dram", bufs=1, space="DRAM") as dram:

            # ---------- static tiles ----------
            ident = sb.tile([128, 128], dt.float32)
            make_identity(nc, ident[:])

            A8 = sb.tile([C, APIX], dt.bfloat16)
            nc.gpsimd.memset(A8[:], 0.0)  # zero borders once; writes stay interior

            # x resident (tag shared with the c1/c2 buffer, which is larger)
            x_sb = sb.tile([C, BS * NPIX_IMG], dt.float32, tag="big", name="x_sb")
            x_src = x_in.ap().rearrange("b c h w -> c b (h w)")
            nc.sync.dma_start(x_sb[:].rearrange("p (b n) -> p b n", b=BS), x_src)

            # small inputs
            def row9(name, t):
                r = sb.tile([1, 9], dt.float32, name=name)
                nc.sync.dma_start(r[:], t.ap()[None, :])
                return r

            p1r, gn1r = row9("p1r", p1_in), row9("gn1r", gn1_in)
            p2r, gn2r = row9("p2r", p2_in), row9("gn2r", gn2_in)
            taur = sb.tile([1, 1], dt.float32)
            nc.sync.dma_start(taur[:], tau_in.ap()[None, :])
            constsr = sb.tile([1, 18], dt.float32)
            nc.sync.dma_start(constsr[:], consts_in.ap())

            def col128(name, t):
                r = sb.tile([C, 1], dt.float32, name=name)
                nc.sync.dma_start(r[:], t.ap()[:, None])
                return r

            gam1, bet1 = col128("gam1", g1_in), col128("bet1", b1_in)
            gam2, bet2 = col128("gam2", g2_in), col128("bet2", b2_in)

            rtau = sb.tile([1, 1], dt.float32)
            nc.vector.reciprocal(rtau[:], taur[:])

            # ---------- per-layer softmax -> alpha/gamma strip -> broadcast ----------
            def softmax_strip(pr, gnr, tag):
                u = sb.tile([1, 9], dt.float32, name=f"u_{tag}")
                nc.vector.tensor_tensor(u[:], pr[:], gnr[:], Alu.add)
                nc.vector.tensor_scalar(u[:], u[:], rtau[:, 0:1], None, Alu.mult)
                mx = sb.tile([1, 1], dt.float32, name=f"mx_{tag}")
                nc.vector.tensor_reduce(mx[:], u[:], axis=mybir.AxisListType.X,
                                        op=Alu.max)
                nmx = sb.tile([1, 1], dt.float32, name=f"nmx_{tag}")
                nc.vector.tensor_scalar(nmx[:], mx[:], -1.0, None, Alu.mult)
                e = sb.tile([1, 9], dt.float32, name=f"e_{tag}")
                nc.scalar.activation(e[:], u[:], Act.Exp, bias=nmx[:, 0:1], scale=1.0)
                ssum = sb.tile([1, 1], dt.float32, name=f"ss_{tag}")
                nc.vector.tensor_reduce(ssum[:], e[:], axis=mybir.AxisListType.X,
                                        op=Alu.add)
                rsum = sb.tile([1, 1], dt.float32, name=f"rs_{tag}")
                nc.vector.reciprocal(rsum[:], ssum[:])
                wrow = sb.tile([1, 9], dt.float32, name=f"w_{tag}")
                nc.vector.tensor_scalar(wrow[:], e[:], rsum[:, 0:1], None, Alu.mult)
                # strip: cols 0-8 alpha_k = w_k * 2/(nw*na); cols 9-11 gamma_g
                strip = sb.tile([1, 12], dt.float32, name=f"strip_{tag}")
                nc.vector.tensor_tensor(strip[:, 0:9], wrow[:], constsr[:, 0:9],
                                        Alu.mult)
                pe1 = sb.tile([1, 9], dt.float32, name=f"pe1_{tag}")
                nc.vector.tensor_tensor(pe1[:], wrow[:], constsr[:, 9:18], Alu.mult)
                pe13 = pe1[:].rearrange("p (i g) -> p i g", g=3)
                for g in range(3):
                    nc.vector.tensor_reduce(strip[:, 9 + g:10 + g], pe13[:, :, g],
                                            axis=mybir.AxisListType.X, op=Alu.add,
                                            negate=True)
                bcast = sb.tile([C, 12], dt.float32, name=f"bcast_{tag}")
                nc.gpsimd.partition_broadcast(bcast[:], strip[:])
                return bcast

            bc1 = softmax_strip(p1r, gn1r, "l1")
            bc2 = softmax_strip(p2r, gn2r, "l2")

            # ---------- weight preparation (two passes over candidates) ----------
            def prep_weights(w_in_t, bcast, tag):
                """Returns Wt[g][h] = [C(i), 9, C(o)] bf16 lhsT tiles.

                Pass 1 computes the per-candidate global |tanh| max (needs all
                nine candidates before any quantization), pass 2 recomputes
                tanh and folds the quantized integer grids into the three
                per-group weight tensors, split hi/lo bf16 and transposed.
                """
                wsrc = w_in_t.ap().rearrange("k o i a b -> k o (i a b)")
                amax = sb.tile([C, 9], dt.float32, name=f"amax_{tag}")
                for k in KORDER:
                    raw = sb.tile([C, 1152], dt.float32, tag="wraw", bufs=2,
                                  name=f"rawa_{tag}_{k}")
                    nc.sync.dma_start(raw[:], wsrc[k])
                    th = sb.tile([C, 1152], dt.float32, tag="wth", bufs=2,
                                 name=f"tha_{tag}_{k}")
                    nc.scalar.activation(th[:], raw[:], Act.Tanh, bias=0.0, scale=1.0)
                    nc.vector.tensor_reduce(amax[:, k:k + 1], th[:],
                                            axis=mybir.AxisListType.X, op=Alu.max,
                                            apply_absolute_value=True)
                amr = sb.tile([C, 9], dt.float32, name=f"amr_{tag}")
                nc.gpsimd.partition_all_reduce(amr[:], amax[:], channels=C,
                                               reduce_op=bass_isa.ReduceOp.max)
                a2 = sb.tile([C, 9], dt.float32, name=f"a2_{tag}")
                nc.vector.tensor_scalar(a2[:], amr[:], 2.0, None, Alu.mult)
                r2 = sb.tile([C, 9], dt.float32, name=f"r2_{tag}")
                nc.vector.reciprocal(r2[:], a2[:])

                Wt = [[None, None] for _ in range(3)]
                for g in range(3):
                    for h in range(2):
                        Wt[g][h] = sb.tile([C, 9, C], dt.bfloat16,
                                           name=f"W_{tag}_{g}_{h}")

                wacc = None
                for idx, k in enumerate(KORDER):
                    g = k % 3
                    raw = sb.tile([C, 1152], dt.float32, tag="wraw", bufs=2,
                                  name=f"rawb_{tag}_{k}")
                    nc.sync.dma_start(raw[:], wsrc[k])
                    th = sb.tile([C, 1152], dt.float32, tag="wth", bufs=2,
                                 name=f"thb_{tag}_{k}")
                    nc.scalar.activation(th[:], raw[:], Act.Tanh, bias=0.0, scale=1.0)
                    # wn = th/(2amax) + 0.5 ; u2 = wn*nw + M (rounds) ; m = u2 - M
                    nc.vector.tensor_scalar(th[:], th[:], r2[:, k:k + 1], 0.5,
                                            Alu.mult, Alu.add)
                    nc.vector.tensor_scalar(th[:], th[:], float(NW[k]), MAGIC,
                                            Alu.mult, Alu.add)
                    nc.vector.tensor_scalar(th[:], th[:], MAGIC, None, Alu.subtract)
                    pos = idx % 3  # position within the group (KORDER is group-major)
                    if pos == 0:
                        wacc = sb.tile([C, 1152], dt.float32, tag="wacc", bufs=2,
                                       name=f"wacc_{tag}_{g}_{pos}")
                        nc.vector.tensor_scalar(wacc[:], th[:], bcast[:, k:k + 1],
                                                bcast[:, 9 + g:10 + g],
                                                Alu.mult, Alu.add)
                    else:
                        nxt = sb.tile([C, 1152], dt.float32, tag="wacc", bufs=2,
                                      name=f"wacc_{tag}_{g}_{pos}")
                        nc.vector.scalar_tensor_tensor(nxt[:], th[:],
                                                       bcast[:, k:k + 1], wacc[:],
                                                       Alu.mult, Alu.add)
                        wacc = nxt
                    if pos == 2:
                        w3 = wacc[:].rearrange("p (i t) -> p i t", t=9)
                        for t in range(9):
                            tp = ps.tile([128, 128], dt.float32, tag="tps", bufs=2,
                                         name=f"tp_{tag}_{g}_{t}")
                            nc.tensor.transpose(tp[:], w3[:, :, t], ident[:])
                            nc.vector.tensor_copy(Wt[g][0][:, t, :], tp[:])
                            nc.vector.tensor_tensor(Wt[g][1][:, t, :], tp[:],
                                                    Wt[g][0][:, t, :], Alu.subtract)
                return Wt

            # ---------- BN scalar math ----------
            def bn_scalars(glob, gam, bet, tag):
                """glob [C,2] global (sum, sumsq) -> (scale_q, bias_q) [C,1]."""
                def t1(name):
                    return sb.tile([C, 1], dt.float32, name=f"{name}_{tag}")
                mean, e2, msq, var, ve = (t1("mean"), t1("e2"), t1("msq"),
                                          t1("var"), t1("ve"))
                nc.vector.tensor_scalar(mean[:], glob[:, 0:1], 1.0 / NTOT, None,
                                        Alu.mult)
                nc.vector.tensor_scalar(e2[:], glob[:, 1:2], 1.0 / NTOT, None,
                                        Alu.mult)
                nc.vector.tensor_tensor(msq[:], mean[:], mean[:], Alu.mult)
                nc.vector.tensor_tensor(var[:], e2[:], msq[:], Alu.subtract)
                nc.vector.tensor_scalar(ve[:], var[:], EPS, None, Alu.add)
                sq, y = t1("sq"), t1("y0")
                nc.scalar.activation(sq[:], ve[:], Act.Sqrt, bias=0.0, scale=1.0)
                nc.vector.reciprocal(y[:], sq[:])
                for it in range(2):  # Newton: y <- y*(1.5 - 0.5*ve*y^2)
                    tt1, tt2, tt3, yn = (t1(f"n{it}a"), t1(f"n{it}b"),
                                         t1(f"n{it}c"), t1(f"y{it + 1}"))
                    nc.vector.tensor_tensor(tt1[:], y[:], y[:], Alu.mult)
                    nc.vector.tensor_tensor(tt2[:], tt1[:], ve[:], Alu.mult)
                    nc.vector.tensor_scalar(tt3[:], tt2[:], -0.5, 1.5, Alu.mult,
                                            Alu.add)
                    nc.vector.tensor_tensor(yn[:], y[:], tt3[:], Alu.mult)
                    y = yn
                sbn, bt, bbn, sq_q, bq_q = (t1("sbn"), t1("bt"), t1("bbn"),
                                            t1("sclq"), t1("biasq"))
                nc.vector.tensor_tensor(sbn[:], gam[:], y[:], Alu.mult)
                nc.vector.tensor_tensor(bt[:], mean[:], sbn[:], Alu.mult)
                nc.vector.tensor_scalar(bbn[:], bt[:], -1.0, bet[:, 0:1], Alu.mult,
                                        Alu.add)
                nc.vector.tensor_scalar(sq_q[:], sbn[:], 255.0, None, Alu.mult)
                nc.vector.tensor_scalar(bq_q[:], bbn[:], 255.0, None, Alu.mult)
                return sq_q, bq_q

            def allreduce_stats(stats_cols, tag):
                """stats_cols [C,8] (4 img sums, 4 img sumsqs) -> glob [C,2]."""
                loc = sb.tile([C, 2], dt.float32, name=f"loc_{tag}")
                sc3 = stats_cols[:].rearrange("p (s i) -> p s i", s=2)
                nc.vector.tensor_reduce(loc[:], sc3, axis=mybir.AxisListType.X,
                                        op=Alu.add)
                cin = dram.tile([C, 2], dt.float32, name=f"ccin_{tag}")
                cout = dram.tile([C, 2], dt.float32, addr_space="Shared",
                                 name=f"ccout_{tag}")
                nc.sync.dma_start(cin[:], loc[:])
                nc.gpsimd.collective_compute(
                    "AllReduce", Alu.add,
                    replica_groups=[list(range(N_CORES))],
                    ins=[cin.opt()], outs=[cout.opt()])
                glob = sb.tile([C, 2], dt.float32, name=f"glob_{tag}")
                nc.sync.dma_start(glob[:], cout[:])
                return glob

            def img_stats(src3d, stats_cols, i, tag):
                """per-image sum/sumsq of [C,56,56] view into cols i, 4+i."""
                scr = sb.tile([C, NPIX_IMG], dt.float32, tag="scr", bufs=2,
                              name=f"scs_{tag}_{i}")
                scr3 = scr[:].rearrange("p (a b) -> p a b", a=H)
                nc.scalar.activation(scr3, src3d, Act.Copy, bias=0.0, scale=1.0,
                                     accum_out=stats_cols[:, i:i + 1])
                scr2 = sb.tile([C, NPIX_IMG], dt.float32, tag="scr", bufs=2,
                               name=f"scq_{tag}_{i}")
                scr23 = scr2[:].rearrange("p (a b) -> p a b", a=H)
                nc.scalar.activation(scr23, src3d, Act.Square, bias=0.0, scale=1.0,
                                     accum_out=stats_cols[:, 4 + i:5 + i])

            # ---------- quantize one image into A8 interior ----------
            def quantize_img(src3d, scale_q, bias_q, i, tag):
                u = sb.tile([C, NPIX_IMG], dt.float32, tag="scr", bufs=2,
                            name=f"qu_{tag}_{i}")
                u3 = u[:].rearrange("p (a b) -> p a b", a=H)
                nc.scalar.activation(u3, src3d, Act.Relu, bias=bias_q[:, 0:1],
                                     scale=scale_q[:, 0:1])
                nc.vector.tensor_scalar(u[:], u[:], 255.0, MAGIC, Alu.min, Alu.add)
                dst = A8[:, i * IMG + BASE:i * IMG + BASE + IMG_SPAN]
                dst3 = dst.rearrange("p (a b) -> p a b", b=WP)[:, :, 0:W]
                nc.vector.tensor_scalar(dst3, u3, MAGIC, None, Alu.subtract)

            # ---------- conv layer ----------
            def conv_layer(Wt, cdst, tag, per_img_done):
                """6 passes x 9 taps x chunks accumulate in PSUM; copy -> cdst.
                per_img_done(i) is emitted after image i's last superchunk."""
                img_last_sc = {}
                for i in range(BS):
                    last = i * IMG + (H * WP + W + 1) - 1  # last valid flat idx
                    s_last = min((last - VSTART) // SLEN, NSC - 1)
                    assert s_last not in img_last_sc
                    img_last_sc[s_last] = i
                for s in range(NSC):
                    chunks = _chunks_of_sc(s)
                    start = VSTART + s * SLEN
                    lo = min(max(start - 64, 0), APIX - STG)
                    a4s = sb.tile([C, STG], dt.bfloat16, tag="a4s", bufs=2,
                                  name=f"a4_{tag}_{s}")
                    nc.vector.tensor_scalar(a4s[:], A8[:, lo:lo + STG], 1.0 / 17.0,
                                            MAGICB, Alu.mult, Alu.add)
                    nc.vector.tensor_scalar(a4s[:], a4s[:], MAGICB, None,
                                            Alu.subtract)
                    a2s = sb.tile([C, STG], dt.bfloat16, tag="a2s", bufs=2,
                                  name=f"a2_{tag}_{s}")
                    nc.vector.tensor_scalar(a2s[:], a4s[:], 1.0 / 5.0, MAGICB,
                                            Alu.mult, Alu.add)
                    nc.vector.tensor_scalar(a2s[:], a2s[:], MAGICB, None,
                                            Alu.subtract)

                    pt = ps.tile([128, SLEN], dt.float32, tag="cps", bufs=2,
                                 name=f"ps_{tag}_{s}")
                    passes = [(2, 0), (2, 1), (1, 0), (1, 1), (0, 0), (0, 1)]
                    for pi, (g, hh) in enumerate(passes):
                        for t in range(9):
                            off = TAPS[t]
                            for (pcol, gs, ln) in chunks:
                                if g == 2:
                                    rhs = A8[:, gs + off:gs + off + ln]
                                elif g == 1:
                                    rhs = a4s[:, gs + off - lo:gs + off - lo + ln]
                                else:
                                    rhs = a2s[:, gs + off - lo:gs + off - lo + ln]
                                nc.tensor.matmul(
                                    pt[:, pcol:pcol + ln], Wt[g][hh][:, t, :], rhs,
                                    start=(pi == 0 and t == 0),
                                    stop=(pi == len(passes) - 1 and t == 8))
                    sc_end = min(start + SLEN, VEND)
                    nc.scalar.copy(cdst[:, start:sc_end], pt[:, 0:sc_end - start])
                    if s in img_last_sc:
                        per_img_done(img_last_sc[s])

            # ================= LAYER 1 =================
            W1 = prep_weights(w1_in, bc1, "w1")

            stats1 = sb.tile([C, 8], dt.float32)
            x3 = x_sb[:].rearrange("p (b a w) -> p b a w", b=BS, a=H)
            for i in range(BS):
                img_stats(x3[:, i], stats1, i, "s1")
            glob1 = allreduce_stats(stats1, "c1")
            sq1, bq1 = bn_scalars(glob1, gam1, bet1, "bn1")
            for i in range(BS):
                quantize_img(x3[:, i], sq1, bq1, i, "q1")

            # c1 reuses x's slot (same tag) — x is dead after quantize-1
            c1 = sb.tile([C, APIX], dt.float32, tag="big", name="c1buf")
            stats2 = sb.tile([C, 8], dt.float32)

            def c1_img3d(i):
                v = c1[:, i * IMG + BASE:i * IMG + BASE + IMG_SPAN]
                return v.rearrange("p (a b) -> p a b", b=WP)[:, :, 0:W]

            conv_layer(W1, c1, "cv1",
                       lambda i: img_stats(c1_img3d(i), stats2, i, "s2"))

            # layer-2 weight prep: emitted after conv1 so the scheduler fills
            # conv1's DVE/ACT idle time with it
            W2 = prep_weights(w2_in, bc2, "w2")

            # ================= LAYER 2 =================
            glob2 = allreduce_stats(stats2, "c2")
            sq2, bq2 = bn_scalars(glob2, gam2, bet2, "bn2")
            for i in range(BS):
                quantize_img(c1_img3d(i), sq2, bq2, i, "q2")

            out_v = out_dram.ap().rearrange("b c h w -> c b (h w)")
            x_src2 = x_in.ap().rearrange("b c h w -> c b (h w)")

            def residual_out(i):
                xr = sb.tile([C, NPIX_IMG], dt.float32, tag="scr", bufs=2,
                             name=f"xr_{i}")
                nc.sync.dma_start(xr[:], x_src2[:, i, :])
                xr3 = xr[:].rearrange("p (a b) -> p a b", a=H)
                nc.vector.tensor_tensor(xr3, xr3, c1_img3d(i), Alu.add)
                nc.sync.dma_start(out_v[:, i, :], xr[:])

            # c2 reuses the same "big" buffer (c1 dead after quantize-2/stats)
            conv_layer(W2, c1, "cv2", residual_out)

    nc.compile()
    return nc


def _consts():
    c = np.zeros((1, 18), np.float32)
    for k in range(9):
        c[0, k] = 2.0 / (NW[k] * NA[k])
        c[0, 9 + k] = 1.0 / NA[k]
    return c


def _in_maps(inputs):
    x = np.ascontiguousarray(inputs["x"], dtype=np.float32)
    shared = {
        "conv1_w": np.ascontiguousarray(inputs["conv1_w"], dtype=np.float32),
        "conv2_w": np.ascontiguousarray(inputs["conv2_w"], dtype=np.float32),
        "gamma1": np.ascontiguousarray(inputs["gamma1"], dtype=np.float32),
        "beta1": np.ascontiguousarray(inputs["beta1"], dtype=np.float32),
        "gamma2": np.ascontiguousarray(inputs["gamma2"], dtype=np.float32),
        "beta2": np.ascontiguousarray(inputs["beta2"], dtype=np.float32),
        "p1": np.ascontiguousarray(inputs["p1"], dtype=np.float32),
        "p2": np.ascontiguousarray(inputs["p2"], dtype=np.float32),
        "gn1": np.ascontiguousarray(inputs["gn1"], dtype=np.float32),
        "gn2": np.ascontiguousarray(inputs["gn2"], dtype=np.float32),
        "tau": np.asarray(inputs["tau"], dtype=np.float32).reshape(1),
        "consts": _consts(),
    }
    return [dict(shared, x=x[c * BS:(c + 1) * BS]) for c in range(N_CORES)]


def _get_nc():
    if "nc" not in _CACHE:
        _CACHE["nc"] = _build()
    return _CACHE["nc"]


def _run(in_maps, trace=False):
    nc = _get_nc()
    return bass_utils.run_bass_kernel_spmd(
        nc, in_maps, core_ids=list(range(N_CORES)), trace=trace)


def kernel(**inputs) -> np.ndarray:
    res = _run(_in_maps(inputs))
    return np.concatenate([res.results[c]["out"] for c in range(N_CORES)], axis=0)
